# revision 13
# baseline (speedup 1.0000x reference)
"""AttentionBlock (B=4, C=256, H=W=64) on 8 Trainium2 NeuronCores.

Sharding: data-parallel over (batch, query-half): core i handles batch i//2,
query pixels [half*2048, (half+1)*2048), half = i%2. GroupNorm stats are
computed per batch element (duplicated across the pair, cheap); the O(N^2)
attention work is fully sharded 8 ways. No collectives.

v5: v4's fp8 DoubleRow attention core (already at the fp8 PE roofline in
steady state) with the serial head rebuilt around the PE:

  - GroupNorm sums come from PE Gram matmuls on xT8e (x^T x per channel
    block with an appended ones column -> sum x and sum x^2 per channel),
    accumulating while the DMA load streams in. This removes the 16-op
    bn_stats chain from DVE and doubles as the PE p-state warmup.
    Diagonal extraction via tensor_tensor_reduce against a host eye mask;
    rstd_g = exp(-0.5*ln(var+eps)) keeps ACT inside one table family
    (ln/exp/identity/copy) so the 1.3us table loads happen once, early.
  - x8h is gone: per-core host prep permutes key blocks so the core's own
    query half occupies x8 columns [0, 2048) -- the q-conv reads x8
    directly and the S/xe key order (attention is key-order invariant)
    follows the same permutation in xT8e.
  - xh (residual) ships as bf16, wq/wvf/wpf as bf16: less ring time, no
    meaningful precision change (biases are small; w8q is fp8 anyway).
  - DMA as few large transfers spread over the 4 DGE rings (~600ns of
    engine queue time each), ordered by criticality; ACT ring stays empty
    so the activation warms run immediately.
  - q-conv epilogues split ACT/DVE; vbias fills the qq-cast gap; at chunk
    boundaries ACT takes one of the two xe8 casts so the next chunk's
    xe(0) sees po freed ~0.6us earlier; last chunk stores split over two
    rings.

Attention core (unchanged from v4):
  S[m,n]  = sum_c x[c,m] * qq[c,n],   qq = s .* (Wk^T q)   (k-conv fused)
  O[o,n]  = sum_c Wv[c,o] * s[c] * xe[c,n],  xe = sum_m x[c,m] E[m,n]
  E = exp(S/16 - 3.0) in fp8 (shift cancels via R; -3.0 keeps the
  data's max S ~138 clear of fp8e4's 448 ceiling); xe accumulated /64-scaled in fp8, the x64
  restored via the R-broadcast; R (softmax denominator) via a DoubleRow
  ones-matmul accumulated alongside xe; 1/R via the fast DVE reciprocal.
  Per 512-query chunk: 16 key-block pairs, software-pipelined so the PE
  executes {S(g+1), xe(g-1), R(g-1)} during exp(g)."""

import numpy as np

B, C, HW = 4, 256, 4096
NH = 2048            # query pixels per core
G, CPG = 32, 8       # groups, channels per group
EPS = 1e-5
MB = HW // 128       # 32 key blocks
NP = MB // 2         # 16 key-block pairs
XW = 288             # xT8e padded row: [x_c0 0:128 | ones 128 | x_c1 144:272 | ones 272]
C1OFF = 144

_cache = {}


def build_nc():
    """Build (and cache) the Bass module."""
    if "nc" in _cache:
        return _cache["nc"]
    import concourse.tile as tile
    from concourse import bacc, mybir

    f32 = mybir.dt.float32
    f32r = mybir.dt.float32r
    fp8 = mybir.dt.float8e4
    bf16 = mybir.dt.bfloat16
    AF = mybir.ActivationFunctionType
    OP = mybir.AluOpType
    DR = mybir.MatmulPerfMode.DoubleRow

    nc = bacc.Bacc("TRN2", target_bir_lowering=False, debug=False,
                   enable_asserts=False, num_devices=8)

    # ---- DRAM I/O (host preps everything into device layout) ----
    d_x8 = nc.dram_tensor("x8", [128, 2, HW], fp8, kind="ExternalInput")
    d_xT8 = nc.dram_tensor("xT8", [128, MB, XW], fp8, kind="ExternalInput")
    d_xh = nc.dram_tensor("xh", [128, 2, NH], bf16, kind="ExternalInput")
    d_wq = nc.dram_tensor("wq", [128, 2, C], bf16, kind="ExternalInput")
    d_wvpf = nc.dram_tensor("wvpf", [128, 2, 2 * C], bf16, kind="ExternalInput")
    d_wkTb = nc.dram_tensor("wkTb", [128, 2, C], bf16, kind="ExternalInput")
    d_wv88 = nc.dram_tensor("wv88", [128, 2, 2 * C], fp8, kind="ExternalInput")
    d_hdr = nc.dram_tensor("hdr", [128, 2, 5 + G], f32, kind="ExternalInput")
    d_bg = nc.dram_tensor("bg", [G, 2, 128], f32, kind="ExternalInput")
    d_eye = nc.dram_tensor("eye", [128, 132], bf16, kind="ExternalInput")
    d_out = nc.dram_tensor("out", [128, 2, NH], f32, kind="ExternalOutput")

    with tile.TileContext(nc) as tc:
        with (
            tc.tile_pool(name="big", bufs=1) as big,
            tc.tile_pool(name="cst", bufs=1) as cst,
            tc.tile_pool(name="wrk", bufs=3) as wrk,
            tc.tile_pool(name="epool", bufs=6) as epool,
            tc.tile_pool(name="gnp", bufs=1) as gnp,
            tc.tile_pool(name="ps_s", bufs=2, space="PSUM") as ps_s,
            tc.tile_pool(name="ps_o", bufs=1, space="PSUM") as ps_o,
            tc.tile_pool(name="ps_t", bufs=1, space="PSUM") as ps_t,
        ):
            # ---- ACT warm: exp then ln on a scratch tile. The act-table
            # pass inserts the (up to two) table loads right here, ~7us,
            # while ACT is otherwise idle; every later activation (ln,
            # exp, identity, copy) hits a loaded table.
            warm = cst.tile([1, 2], f32, tag="warm")
            nc.vector.memset(warm, 1.0)
            nc.scalar.activation(out=warm[:, 0:1], in_=warm[:, 0:1],
                                 func=AF.Exp)
            nc.scalar.activation(out=warm[:, 1:2], in_=warm[:, 1:2],
                                 func=AF.Sqrt)

            # ---- input loads. Per-dispatch engine-queue cost is ~600ns,
            # so: few, large transfers over the 3 DGE rings (SP/ACT/Pool),
            # ordered by criticality: xT8e (stats) on sync+gpsimd, own-half
            # x8 (q-conv) early on the ACT ring, trailing bulk last.
            hdr = cst.tile([128, 2, 5 + G], f32, tag="hdr")
            nc.gpsimd.dma_start(out=hdr, in_=d_hdr.ap())
            bg = cst.tile([G, 2, 128], f32, tag="bg")
            nc.gpsimd.dma_start(out=bg, in_=d_bg.ap())
            eye = cst.tile([128, 132], bf16, tag="eye")
            nc.gpsimd.dma_start(out=eye, in_=d_eye.ap())
            qb = hdr[:, :, 0:1]
            gb = hdr[:, :, 3:4]
            rbias = hdr[:, :, 4:5]
            ag = hdr[:, :, 5:5 + G]

            # x8: own query half (cols 0:2048 after the host block
            # permutation) first -- feeds q-conv; S pairs then consume
            # blocks in permuted order, covered by the halves.
            x8 = big.tile([128, 2, HW], fp8, tag="x8")
            for q4 in range(2):
                for ci in range(2):
                    nc.scalar.dma_start(
                        out=x8[:, ci, q4 * 1024:(q4 + 1) * 1024],
                        in_=d_x8.ap()[:, ci, q4 * 1024:(q4 + 1) * 1024])
            xT8 = big.tile([128, MB, XW], fp8, tag="xT8")
            for q4 in range(4):
                eng = nc.gpsimd if (q4 % 2 == 0) else nc.sync
                eng.dma_start(out=xT8[:, q4 * 8:(q4 + 1) * 8, :],
                              in_=d_xT8.ap()[:, q4 * 8:(q4 + 1) * 8, :])
            wq = cst.tile([128, 2, C], bf16, tag="wq")
            nc.sync.dma_start(out=wq, in_=d_wq.ap())
            wkTb = cst.tile([128, 2, C], bf16, tag="wkTb")
            nc.scalar.dma_start(out=wkTb, in_=d_wkTb.ap())
            wvpf = cst.tile([128, 2, 2 * C], bf16, tag="wvpf")
            nc.scalar.dma_start(out=wvpf, in_=d_wvpf.ap())
            wv88 = cst.tile([128, 2, 2 * C], fp8, tag="wv88")
            nc.scalar.dma_start(out=wv88, in_=d_wv88.ap())
            wv8 = wv88[:, :, 0:C]
            wp8 = wv88[:, :, C:2 * C]
            for q4 in range(2, 4):   # other key half, needed from pair 8
                for ci in range(2):
                    nc.gpsimd.dma_start(
                        out=x8[:, ci, q4 * 1024:(q4 + 1) * 1024],
                        in_=d_x8.ap()[:, ci, q4 * 1024:(q4 + 1) * 1024])
            xh = big.tile([128, 2, NH], bf16, tag="xh")
            for ci in range(2):
                nc.sync.dma_start(out=xh[:, ci, :], in_=d_xh.ap()[:, ci, :])

            # constants (DVE, tiny)
            epst = cst.tile([G, 1], f32, tag="epst")
            nc.vector.memset(epst, EPS)
            ones21t = cst.tile([128, 2, 16], fp8, tag="ones21")
            nc.vector.memset(ones21t, 1.0)
            ones21 = ones21t[:, :, 0:1]    # R lhsT (DR)
            negc = cst.tile([128, 1], f32, tag="negc")  # exp shift
            nc.vector.memset(negc, -3.0)
            r64s = cst.tile([1, 128], f32, tag="r64s")
            nc.vector.memset(r64s, 64.0)
            row64 = cst.tile([1, 128], f32r, tag="row64")   # 64/R bcast lhsT
            nc.vector.tensor_copy(out=row64, in_=r64s)
            inv64 = cst.tile([128, 1], f32, tag="inv64")
            nc.vector.memset(inv64, 1.0 / 64.0)

            # ---- GroupNorm sums via PE Gram on xT8 (DMA-paced; also the
            # PE warmup). G[:, ci, c'] = sum_m x[c,m] x[c',m]; col 128 =
            # sum_m x[c,m] (ones column).
            # two accumulators in DIFFERENT PSUM banks (an accumulation
            # group owns its 2KB zero region): ci0 in ps_t, ci1 borrows
            # ps_o (idle until chunk 0's po).
            gp0 = ps_t.tile([128, 132], f32, tag="t", name="gram0")
            gp1 = ps_o.tile([128, 132], f32, tag="o", name="gram1")
            gps = [gp0, gp1]
            for p in range(NP):
                for ci in range(2):
                    off = 0 if ci == 0 else C1OFF
                    nc.tensor.matmul(
                        gps[ci][:, 0:132],
                        lhsT=xT8[:, 2 * p:2 * p + 2, off:off + 128],
                        rhs=xT8[:, 2 * p:2 * p + 2, off:off + 132],
                        start=(p == 0), stop=(p == NP - 1), perf_mode=DR)

            # stats: st2[:, ci, 0] = sum x (ag carries 1/(CPG*HW)),
            # st2[:, ci, 1] = sum x^2 (diag of the Gram block).
            scr = gnp.tile([128, 2, 132], f32, tag="scr")
            st2 = gnp.tile([128, 2, 2], f32, tag="st2")
            bst = gnp.tile([128, 2, 6], f32, tag="bst")
            ba2 = gnp.tile([128, 2, 2], f32, tag="ba2")
            for ci in range(2):
                nc.vector.tensor_copy(out=st2[:, ci, 0:1],
                                      in_=gps[ci][:, 128:129])
                # diag extract: eye diagonal carries 129.0, so the mean
                # over the 129 columns of G*eye is exactly diag(G).
                nc.vector.tensor_tensor(
                    out=scr[:, ci, 0:129], in0=gps[ci][:, 0:129],
                    in1=eye[:, 0:129], op=OP.mult)
                nc.vector.bn_stats(out=bst[:, ci, :], in_=scr[:, ci, 0:129])
                nc.vector.bn_aggr(out=ba2[:, ci, :], in_=bst[:, ci, :])
                nc.vector.tensor_copy(out=st2[:, ci, 1:2],
                                      in_=ba2[:, ci, 0:1])
            pg = ps_t.tile([G, 2], f32, tag="t")
            for ci in range(2):
                nc.tensor.matmul(pg, lhsT=ag[:, ci, :], rhs=st2[:, ci, :],
                                 start=(ci == 0), stop=(ci == 1))
            pgs = gnp.tile([G, 2], f32, tag="pgs")
            nc.vector.tensor_copy(out=pgs, in_=pg)
            gst = gnp.tile([G, 4], f32, tag="gst")
            nc.vector.tensor_tensor(out=gst[:, 0:1], in0=pgs[:, 0:1],
                                    in1=pgs[:, 0:1], op=OP.mult)
            nc.vector.tensor_tensor(out=gst[:, 1:2], in0=pgs[:, 1:2],
                                    in1=gst[:, 0:1], op=OP.subtract)
            # rstd_g = exp(-0.5*ln(var+eps)): stays in the loaded tables
            gfin = gnp.tile([G, 2], f32, tag="gfin")  # (rstd_g, mean_g*rstd_g)
            # NOTE: Ln on ACT hard-crashes this HW (NRT_EXEC_UNIT_
            # UNRECOVERABLE) -- use Sqrt + DVE reciprocal. Costs one extra
            # act-table load (sqrt set vs exp set), both warmed early.
            nc.scalar.activation(out=gst[:, 2:3], in_=gst[:, 1:2],
                                 func=AF.Sqrt, bias=epst)
            nc.vector.reciprocal(out=gfin[:, 0:1], in_=gst[:, 2:3])
            nc.vector.tensor_tensor(out=gfin[:, 1:2], in0=pgs[:, 0:1],
                                    in1=gfin[:, 0:1], op=OP.mult)
            # bg carries gn_w: pbc = (scale_c, mean_c*scale_c);
            # bias_c = gn_b - mean_c*scale_c
            scbc = gnp.tile([128, 2, 2], f32, tag="scbc")
            for ci in range(2):
                pbc = ps_t.tile([128, 2], f32, tag="t")
                nc.tensor.matmul(pbc, lhsT=bg[:, ci, :], rhs=gfin,
                                 start=True, stop=True)
                nc.vector.tensor_copy(out=scbc[:, ci, 0:1], in_=pbc[:, 0:1])
                nc.vector.tensor_tensor(out=scbc[:, ci, 1:2], in0=gb[:, ci, :],
                                        in1=pbc[:, 1:2], op=OP.subtract)

            # q weights: fold GN scale, cast fp8 (q-conv is the only conv)
            w8q = cst.tile([128, 2, C], fp8, tag="w8q")
            for ci in range(2):
                nc.vector.tensor_scalar(
                    out=w8q[:, ci, :], in0=wq[:, ci, :],
                    scalar1=scbc[:, ci, 0:1], scalar2=None, op0=OP.mult)
            # per-partition scale for the ACT-side xe8 cast at boundaries
            sc64 = gnp.tile([128, 2, 1], f32, tag="sc64")
            nc.vector.tensor_scalar(
                out=sc64, in0=scbc[:, :, 0:1], scalar1=inv64, scalar2=None,
                op0=OP.mult)

            # bias chain:
            #   bias2q = qb + Wq^T bias_c         (per q out-channel)
            bcc = cst.tile([128, 2, 2], bf16, tag="bcc")
            for ci in range(2):
                nc.vector.tensor_copy(out=bcc[:, ci, 0:1], in_=scbc[:, ci, 1:2])
                nc.vector.tensor_copy(out=bcc[:, ci, 1:2], in_=scbc[:, ci, 1:2])
            bias2q = gnp.tile([128, 2, 1], f32, tag="bias2q")
            for cb in range(2):
                pbias = ps_t.tile([128, 2], f32, tag="t")
                for ci in range(2):
                    nc.tensor.matmul(
                        pbias, lhsT=wq[:, ci, cb * 128:(cb + 1) * 128],
                        rhs=bcc[:, ci, :], start=(ci == 0), stop=(ci == 1))
                nc.vector.tensor_tensor(
                    out=bias2q[:, cb, :], in0=pbias[:, 0:1],
                    in1=qb[:, cb, :], op=OP.add)

            # ---- q conv (fp8 DR) -> qt bf16; epilogues split DVE/ACT so
            # the t=0 pair finishes in one epilogue-latency. qt split by t
            # so chunk 0's qq gates on the two t=0 epilogues only.
            qts = [big.tile([128, 2, 2, 512], bf16, tag=f"qt{t}",
                            name=f"qt{t}") for t in range(2)]

            def q_epi(pq, cb, t):
                if cb == 0:
                    nc.vector.tensor_scalar(
                        out=qts[t][:, cb, :, :], in0=pq,
                        scalar1=bias2q[:, cb, :], scalar2=None, op0=OP.add)
                else:
                    nc.scalar.activation(
                        out=qts[t][:, cb, :, :], in_=pq,
                        func=AF.Identity, bias=bias2q[:, cb, :])

            qtiles = []
            for n in range(4):
                cb, t = n % 2, n // 2
                pq = ps_s.tile([128, 2, 512], f32, tag="s")
                for i in range(2):
                    j = 2 * t + i
                    nc.tensor.matmul(
                        pq[:, i, :], lhsT=w8q[:, :, cb * 128:(cb + 1) * 128],
                        rhs=x8[:, :, j * 512:(j + 1) * 512],
                        start=True, stop=True, perf_mode=DR)
                qtiles.append((pq, cb, t))
                if n >= 1:
                    q_epi(*qtiles[n - 1])
            q_epi(*qtiles[3])

            # ---- qq = s .* (Wk^T q): only chunk 0 up front; chunks 1..3
            # are produced inside the preceding chunk's pair loop (ps_t).
            qq8s = [big.tile([128, 2, 512], fp8, tag=f"qq8_{j}",
                             name=f"qq8_{j}") for j in range(4)]

            def qq_ci(j, ci):
                pqq = ps_t.tile([128, 512], f32, tag="t")
                for ch in range(2):
                    nc.tensor.matmul(
                        pqq, lhsT=wkTb[:, ch, ci * 128:(ci + 1) * 128],
                        rhs=qts[j // 2][:, ch, j % 2, :],
                        start=(ch == 0), stop=(ch == 1))
                nc.vector.tensor_scalar(
                    out=qq8s[j][:, ci, :], in0=pqq,
                    scalar1=scbc[:, ci, 0:1], scalar2=None, op0=OP.mult)

            qq_ci(0, 0)
            qq_ci(0, 1)

            # v/proj bias chain (tiny matmuls) -- emitted right here so the
            # PE runs them during the qq-cast wait:
            #   vb2 = Wv^T bias_c ; ub = Wp^T vb2 ; rbias2 = rbias + ub
            vb2pr = gnp.tile([128, 2, 2], bf16, tag="vb2pr")
            rbias2 = gnp.tile([128, 2, 1], f32, tag="rbias2")
            for cb in range(2):
                pvb = ps_t.tile([128, 2], f32, tag="t")
                for ci in range(2):
                    nc.tensor.matmul(
                        pvb, lhsT=wvpf[:, ci, cb * 128:(cb + 1) * 128],
                        rhs=bcc[:, ci, :], start=(ci == 0), stop=(ci == 1))
                nc.vector.tensor_copy(out=vb2pr[:, cb, :], in_=pvb)
            for cb in range(2):
                pub = ps_t.tile([128, 2], f32, tag="t")
                for ch in range(2):
                    nc.tensor.matmul(
                        pub, lhsT=wvpf[:, ch, C + cb * 128:C + (cb + 1) * 128],
                        rhs=vb2pr[:, ch, :], start=(ch == 0),
                        stop=(ch == 1))
                nc.vector.tensor_tensor(
                    out=rbias2[:, cb, :], in0=pub[:, 0:1],
                    in1=rbias[:, cb, :], op=OP.add)

            # residual-with-bias tile; the adds are emitted inside chunk 0's
            # pair loop (DVE is idle there) -- needed first at tail(0)
            xo = big.tile([128, 2, NH], f32, tag="xo")

            def xo_step(n):
                cb, jj = n // 2, n % 2
                sl1 = slice(jj * 1024, (jj + 1) * 1024)
                nc.vector.tensor_scalar(
                    out=xo[:, cb, sl1], in0=xh[:, cb, sl1],
                    scalar1=rbias2[:, cb, :], scalar2=None, op0=OP.add)

            # ---- attention (pipelined within chunks AND across chunk
            # boundaries: the next chunk's first two S-pairs are emitted
            # before this chunk's tail so the exp stream never drains) ----
            NJ = NH // 512
            stiles = [[None] * NP for _ in range(NJ)]
            ets = [[None] * NP for _ in range(NJ)]

            def s_pair(j, g):
                st = ps_s.tile([128, 2, 512], f32, tag="s")
                stiles[j][g] = st
                for i in range(2):
                    mb = 2 * g + i
                    nc.tensor.matmul(
                        st[:, i, :],
                        lhsT=x8[:, :, mb * 128:(mb + 1) * 128],
                        rhs=qq8s[j], start=True, stop=True,
                        perf_mode=DR)

            def tail_head(po, on_act=False):
                """Free po fast: the two xe8 casts go one to DVE, one to
                ACT (idle at the boundary until the next chunk's first S
                lands), the reciprocal chain on DVE."""
                rinv = wrk.tile([1, 512], f32r, tag="rinv")
                xe8 = wrk.tile([128, 2, 512], fp8, tag="xe8")
                nc.vector.tensor_scalar(
                    out=xe8[:, 0, :], in0=po[:, 0, :],
                    scalar1=scbc[:, 0, 0:1], scalar2=inv64,
                    op0=OP.mult, op1=OP.mult)
                nc.scalar.activation(
                    out=xe8[:, 1, :], in_=po[:, 1, :],
                    func=AF.Identity, scale=sc64[:, 1, :])
                rinvf = wrk.tile([1, 512], f32, tag="rinvf")
                nc.vector.reciprocal_approx_fast(out=rinvf, in_=po[0:1, 2, :])
                if on_act:
                    nc.scalar.copy(out=rinv, in_=rinvf)
                else:
                    nc.vector.tensor_copy(out=rinv, in_=rinvf)
                return xe8, rinv

            def tail_steps(sl, xe8, rinv, po=None):
                """O = Wv^T xe8, proj, 64/R renorm, residual add, store.
                Deferred mode (po=None): all PSUM through ps_t (serial
                per-tile WAR chain), steps spread over the next chunk's
                pair loop so nothing blocks the in-order PE queue.
                Last-chunk mode (po given): use po's freed banks so the
                two Wv matmuls and the broadcast run without WAR stalls."""
                st = {}

                def wv(co):
                    def f():
                        if po is None:
                            pw = ps_t.tile([128, 512], f32, tag="t")
                        else:
                            pw = po[:, co, :]
                        st[("w", co)] = pw
                        nc.tensor.matmul(
                            pw, lhsT=wv8[:, :, co * 128:(co + 1) * 128],
                            rhs=xe8, start=True, stop=True, perf_mode=DR,
                            skip_group_check=True)
                    return f

                onorm = wrk.tile([128, 2, 512], fp8, tag="onorm")

                def onrm(co):
                    def f():
                        nc.vector.tensor_copy(out=onorm[:, co, :],
                                              in_=st[("w", co)])
                    return f

                def onrm_both():
                    nc.vector.tensor_copy(out=onorm, in_=po[:, 0:2, :])

                rb = wrk.tile([128, 512], f32, tag="rb")

                def bcast():
                    if po is None:
                        pbx = ps_t.tile([128, 512], f32, tag="t")
                    else:
                        pbx = po[:, 2, :]
                    nc.tensor.matmul(pbx, lhsT=row64, rhs=rinv,
                                     start=True, stop=True,
                                     skip_group_check=True)
                    nc.vector.tensor_copy(out=rb, in_=pbx)

                def proj(co, split=False):
                    def f():
                        if po is None or co == 0:
                            pp = ps_t.tile([128, 512], f32, tag="t",
                                           name="pp")
                        else:
                            pp = po[:, 1, :]
                        nc.tensor.matmul(
                            pp, lhsT=wp8[:, :, co * 128:(co + 1) * 128],
                            rhs=onorm, start=True, stop=True, perf_mode=DR,
                            skip_group_check=True)
                        outt = wrk.tile([128, 512], f32, tag="outt")
                        if not split:
                            nc.vector.tensor_tensor(out=outt, in0=pp, in1=rb,
                                                    op=OP.mult)
                            nc.vector.tensor_tensor(out=outt, in0=outt,
                                                    in1=xo[:, co, sl],
                                                    op=OP.add)
                            nc.sync.dma_start(out=d_out.ap()[:, co, sl],
                                              in_=outt)
                        else:
                            # last chunk: halves, stores on two rings so
                            # the final drain isn't one serial DMA chain
                            for h, eng in ((0, nc.sync), (1, nc.gpsimd)):
                                hs = slice(h * 256, (h + 1) * 256)
                                osl = slice(sl.start + h * 256,
                                            sl.start + (h + 1) * 256)
                                nc.vector.tensor_tensor(
                                    out=outt[:, hs], in0=pp[:, hs],
                                    in1=rb[:, hs], op=OP.mult)
                                nc.vector.tensor_tensor(
                                    out=outt[:, hs], in0=outt[:, hs],
                                    in1=xo[:, co, osl], op=OP.add)
                                eng.dma_start(out=d_out.ap()[:, co, osl],
                                              in_=outt[:, hs])
                    return f

                if po is None:
                    return [wv(0), wv(1), onrm(0), onrm(1), bcast,
                            proj(0), proj(1)]
                return [wv(0), wv(1), onrm_both, bcast,
                        proj(0, split=True), proj(1, split=True)]

            pending = []
            s_pair(0, 0)
            s_pair(0, 1)
            for j in range(NJ):
                sl = slice(j * 512, (j + 1) * 512)
                po = ps_o.tile([128, 3, 512], f32, tag="o")  # xe c0, xe c1, R

                def xe_r(g, et, po=po):
                    for cb in range(2):
                        off = 0 if cb == 0 else C1OFF
                        nc.tensor.matmul(
                            po[:, cb, :],
                            lhsT=xT8[:, 2 * g:2 * g + 2, off:off + 128],
                            rhs=et, start=(g == 0), stop=(g == NP - 1),
                            perf_mode=DR, skip_group_check=True)
                    nc.tensor.matmul(
                        po[0:1, 2, :], lhsT=ones21, rhs=et,
                        start=(g == 0), stop=(g == NP - 1),
                        perf_mode=DR, skip_group_check=True)

                for g in range(NP):
                    et = epool.tile([128, 2, 512], fp8, tag="et")
                    ets[j][g] = et
                    nc.scalar.activation(out=et, in_=stiles[j][g], func=AF.Exp,
                                         scale=1.0 / 16.0, bias=negc)
                    # next chunk's qq early: fills the PE window while
                    # xe(0) still waits for the tail-head to release po
                    if j + 1 < NJ and g in (0, 1):
                        qq_ci(j + 1, g)
                    if g >= 1:
                        xe_r(g - 1, ets[j][g - 1])
                    if g + 2 <= NP - 1:
                        s_pair(j, g + 2)
                    # previous chunk's deferred tail, one step per iteration
                    # starting at g=2 (each step's deps are then ready and
                    # never stall the in-order PE queue)
                    if g >= 2 and pending:
                        pending.pop(0)()
                    # residual prep, spread over chunk 0 (DVE idle here)
                    if j == 0 and 2 <= g <= 5:
                        xo_step(g - 2)
                xe_r(NP - 1, ets[j][NP - 1])
                # next chunk's S warmup precedes this chunk's tail-head so
                # the exp stream never drains across the boundary
                if j + 1 < NJ:
                    xe8, rinv = tail_head(po)
                    s_pair(j + 1, 0)
                    s_pair(j + 1, 1)
                    pending = tail_steps(sl, xe8, rinv)
                else:
                    xe8, rinv = tail_head(po, on_act=True)
                    for f in tail_steps(sl, xe8, rinv, po=po):
                        f()

    nc.compile()
    _cache["nc"] = nc
    return nc


def _prep_maps(x, gn_w, gn_b, qkv_w, qkv_b, proj_w, proj_b):
    """Host-side sharding + layout prep. Returns list of 8 in_maps."""
    import ml_dtypes
    fp8 = ml_dtypes.float8_e4m3
    bf16 = ml_dtypes.bfloat16
    x = np.asarray(x, np.float32)
    qkv_w = np.asarray(qkv_w, np.float32)
    qkv_b = np.asarray(qkv_b, np.float32)
    proj_w = np.asarray(proj_w, np.float32)
    proj_b = np.asarray(proj_b, np.float32)
    gn_w = np.asarray(gn_w, np.float32)
    gn_b = np.asarray(gn_b, np.float32)

    def chunked(a):  # [256, ...] -> [128, 2, ...]
        return np.ascontiguousarray(a.reshape(2, 128, *a.shape[1:]).transpose(
            1, 0, *range(2, a.ndim + 1)))

    wq = chunked(qkv_w[0:C].T.copy()).astype(bf16)       # [c_in, 2, c_out]
    wvf = chunked(qkv_w[2 * C:3 * C].T.copy())           # [c_in, 2, c_out]
    wpf = chunked(proj_w.T.copy())                       # [c_in, 2, c_out]
    wvpf = np.concatenate([wvf, wpf], axis=2).astype(bf16)
    wkTb = chunked(qkv_w[C:2 * C].copy()).astype(bf16)   # [c_out, 2, c_in]
    wv88 = np.concatenate([wvf, wpf], axis=2).astype(fp8)
    rbias = proj_w @ qkv_b[2 * C:3 * C] + proj_b   # v-bias fold + proj bias
    kb_unused = np.zeros(C, np.float32)
    smalls = np.stack([qkv_b[0:C], kb_unused, gn_w, gn_b, rbias], axis=1)

    cidx = np.arange(C)
    ag_full = (cidx[:, None] // CPG == np.arange(G)[None, :]).astype(np.float32)
    ag = ag_full / (CPG * HW)                       # carries 1/(8*4096)
    hdr = chunked(np.concatenate([smalls, ag], axis=1))  # [128, 2, 37]
    bg_full = ag_full * gn_w[:, None]               # carries gn_w
    bg = np.ascontiguousarray(
        bg_full.reshape(2, 128, G).transpose(2, 0, 1))  # [G, 2, 128]
    eye = np.zeros((128, 132), np.float32)
    eye[np.arange(128), np.arange(128)] = 129.0
    eye = eye.astype(bf16)

    maps = []
    for core in range(8):
        b, half = core // 2, core % 2
        xf = x[b].reshape(C, HW)
        if half == 1:   # own query half first (key order is irrelevant
            xf = np.concatenate([xf[:, NH:], xf[:, :NH]], axis=1)
        xf = np.ascontiguousarray(xf)
        xhm = xf[:, 0:NH]                            # own half = residual
        x8c = chunked(xf).astype(fp8)
        # xT8e: [pix128, MB, XW]: [x_c0 | ones | pad | x_c1 | ones | pad]
        xT = np.ascontiguousarray(
            xf.T.reshape(MB, 128, C).transpose(1, 0, 2)).astype(fp8)
        xT8e = np.zeros((128, MB, XW), fp8)
        xT8e[:, :, 0:128] = xT[:, :, 0:128]
        xT8e[:, :, 128] = np.float32(1.0)
        xT8e[:, :, C1OFF:C1OFF + 128] = xT[:, :, 128:256]
        xT8e[:, :, C1OFF + 128] = np.float32(1.0)
        maps.append({
            "x8": x8c,
            "xT8": xT8e, "xh": chunked(xhm).astype(bf16),
            "wq": wq, "wvpf": wvpf,
            "wkTb": wkTb, "wv88": wv88,
            "hdr": hdr, "bg": bg, "eye": eye,
        })
    return maps


def kernel(x, gn_w, gn_b, qkv_w, qkv_b, proj_w, proj_b):
    import concourse.bass_utils as bu
    nc = build_nc()
    maps = _prep_maps(x, gn_w, gn_b, qkv_w, qkv_b, proj_w, proj_b)
    res = bu.run_bass_kernel_spmd(nc, maps, core_ids=list(range(8)))
    out = np.empty((B, C, HW), np.float32)
    for core in range(8):
        b, half = core // 2, core % 2
        o = res.results[core]["out"]                # [128, 2, NH]
        out[b, :, half * NH:(half + 1) * NH] = \
            o.transpose(1, 0, 2).reshape(C, NH)
    return out.reshape(B, C, 64, 64)


# revision 14
# speedup vs baseline: 1.0393x; 1.0393x over previous
"""AttentionBlock (B=4, C=256, H=W=64) on 8 Trainium2 NeuronCores.

Sharding: data-parallel over (batch, query-half): core i handles batch i//2,
query pixels [half*2048, (half+1)*2048), half = i%2. GroupNorm stats are
computed per batch element (duplicated across the pair, cheap); the O(N^2)
attention work is fully sharded 8 ways. No collectives.

v5: v4's fp8 DoubleRow attention core (already at the fp8 PE roofline in
steady state) with the serial head rebuilt around the PE:

  - GroupNorm sums come from PE Gram matmuls on xT8e (x^T x per channel
    block with an appended ones column -> sum x and sum x^2 per channel),
    accumulating while the DMA load streams in. This removes the 16-op
    bn_stats chain from DVE and doubles as the PE p-state warmup.
    Diagonal extraction via tensor_tensor_reduce against a host eye mask;
    rstd_g = exp(-0.5*ln(var+eps)) keeps ACT inside one table family
    (ln/exp/identity/copy) so the 1.3us table loads happen once, early.
  - x8h is gone: per-core host prep permutes key blocks so the core's own
    query half occupies x8 columns [0, 2048) -- the q-conv reads x8
    directly and the S/xe key order (attention is key-order invariant)
    follows the same permutation in xT8e.
  - xh (residual) ships as bf16, wq/wvf/wpf as bf16: less ring time, no
    meaningful precision change (biases are small; w8q is fp8 anyway).
  - DMA as few large transfers spread over the 4 DGE rings (~600ns of
    engine queue time each), ordered by criticality; ACT ring stays empty
    so the activation warms run immediately.
  - q-conv epilogues split ACT/DVE; vbias fills the qq-cast gap; at chunk
    boundaries ACT takes one of the two xe8 casts so the next chunk's
    xe(0) sees po freed ~0.6us earlier; last chunk stores split over two
    rings.

Attention core (unchanged from v4):
  S[m,n]  = sum_c x[c,m] * qq[c,n],   qq = s .* (Wk^T q)   (k-conv fused)
  O[o,n]  = sum_c Wv[c,o] * s[c] * xe[c,n],  xe = sum_m x[c,m] E[m,n]
  E = exp(S/16 - 3.0) in fp8 (shift cancels via R; -3.0 keeps the
  data's max S ~138 clear of fp8e4's 448 ceiling); xe accumulated /64-scaled in fp8, the x64
  restored via the R-broadcast; R (softmax denominator) via a DoubleRow
  ones-matmul accumulated alongside xe; 1/R via the fast DVE reciprocal.
  Per 512-query chunk: 16 key-block pairs, software-pipelined so the PE
  executes {S(g+1), xe(g-1), R(g-1)} during exp(g)."""

import numpy as np

B, C, HW = 4, 256, 4096
NH = 2048            # query pixels per core
G, CPG = 32, 8       # groups, channels per group
EPS = 1e-5
MB = HW // 128       # 32 key blocks
NP = MB // 2         # 16 key-block pairs
XW = 288             # xT8e padded row: [x_c0 0:128 | ones 128 | x_c1 144:272 | ones 272]
C1OFF = 144

_cache = {}


def build_nc():
    """Build (and cache) the Bass module."""
    if "nc" in _cache:
        return _cache["nc"]
    import concourse.tile as tile
    from concourse import bacc, mybir

    f32 = mybir.dt.float32
    f32r = mybir.dt.float32r
    fp8 = mybir.dt.float8e4
    bf16 = mybir.dt.bfloat16
    AF = mybir.ActivationFunctionType
    OP = mybir.AluOpType
    DR = mybir.MatmulPerfMode.DoubleRow

    nc = bacc.Bacc("TRN2", target_bir_lowering=False, debug=False,
                   enable_asserts=False, num_devices=8)

    # ---- DRAM I/O (host preps everything into device layout) ----
    d_x8 = nc.dram_tensor("x8", [128, 2, HW], fp8, kind="ExternalInput")
    d_xT8 = nc.dram_tensor("xT8", [128, MB, XW], fp8, kind="ExternalInput")
    d_xh = nc.dram_tensor("xh", [128, 2, NH], bf16, kind="ExternalInput")
    d_wq = nc.dram_tensor("wq", [128, 2, C], bf16, kind="ExternalInput")
    d_wvpf = nc.dram_tensor("wvpf", [128, 2, 2 * C], bf16, kind="ExternalInput")
    d_wkTb = nc.dram_tensor("wkTb", [128, 2, C], bf16, kind="ExternalInput")
    d_wv88 = nc.dram_tensor("wv88", [128, 2, 2 * C], fp8, kind="ExternalInput")
    d_hdr = nc.dram_tensor("hdr", [128, 2, 5 + G], f32, kind="ExternalInput")
    d_bg = nc.dram_tensor("bg", [G, 2, 128], f32, kind="ExternalInput")
    d_eye = nc.dram_tensor("eye", [128, 132], bf16, kind="ExternalInput")
    d_out = nc.dram_tensor("out", [128, 2, NH], f32, kind="ExternalOutput")

    with tile.TileContext(nc) as tc:
        with (
            tc.tile_pool(name="big", bufs=1) as big,
            tc.tile_pool(name="cst", bufs=1) as cst,
            tc.tile_pool(name="wrk", bufs=3) as wrk,
            tc.tile_pool(name="epool", bufs=6) as epool,
            tc.tile_pool(name="gnp", bufs=1) as gnp,
            tc.tile_pool(name="ps_s", bufs=2, space="PSUM") as ps_s,
            tc.tile_pool(name="ps_o", bufs=1, space="PSUM") as ps_o,
            tc.tile_pool(name="ps_t", bufs=1, space="PSUM") as ps_t,
        ):
            # ---- ACT warm: exp then ln on a scratch tile. The act-table
            # pass inserts the (up to two) table loads right here, ~7us,
            # while ACT is otherwise idle; every later activation (ln,
            # exp, identity, copy) hits a loaded table.
            warm = cst.tile([1, 2], f32, tag="warm")
            nc.vector.memset(warm, 1.0)
            nc.scalar.activation(out=warm[:, 0:1], in_=warm[:, 0:1],
                                 func=AF.Exp)
            nc.scalar.activation(out=warm[:, 1:2], in_=warm[:, 1:2],
                                 func=AF.Sqrt)

            # ---- input loads. Per-dispatch engine-queue cost is ~600ns,
            # so: few, large transfers over the 3 DGE rings (SP/ACT/Pool),
            # ordered by criticality: xT8e (stats) on sync+gpsimd, own-half
            # x8 (q-conv) early on the ACT ring, trailing bulk last.
            hdr = cst.tile([128, 2, 5 + G], f32, tag="hdr")
            nc.gpsimd.dma_start(out=hdr, in_=d_hdr.ap())
            bg = cst.tile([G, 2, 128], f32, tag="bg")
            nc.gpsimd.dma_start(out=bg, in_=d_bg.ap())
            eye = cst.tile([128, 132], bf16, tag="eye")
            nc.gpsimd.dma_start(out=eye, in_=d_eye.ap())
            qb = hdr[:, :, 0:1]
            gb = hdr[:, :, 3:4]
            rbias = hdr[:, :, 4:5]
            ag = hdr[:, :, 5:5 + G]

            # x8: own query half (cols 0:2048 after the host block
            # permutation) first -- feeds q-conv; S pairs then consume
            # blocks in permuted order, covered by the halves.
            # NOTE: one transfer rides ONE DMA engine (~22.5 GB/s), so
            # bandwidth needs MANY in-flight transfers: ~64-148KB slices,
            # dispatched round-robin (each costs ~600ns of engine queue).
            x8 = big.tile([128, 2, HW], fp8, tag="x8")
            for q4 in range(2):      # own query half: q-conv + S pairs 0-7
                for ci in range(2):
                    nc.scalar.dma_start(
                        out=x8[:, ci, q4 * 1024:(q4 + 1) * 1024],
                        in_=d_x8.ap()[:, ci, q4 * 1024:(q4 + 1) * 1024])
            xT8 = big.tile([128, MB, XW], fp8, tag="xT8")
            for g4 in range(8):      # 4-block groups (148KB), 2 rings
                eng = nc.gpsimd if (g4 % 2 == 0) else nc.sync
                eng.dma_start(out=xT8[:, g4 * 4:(g4 + 1) * 4, :],
                              in_=d_xT8.ap()[:, g4 * 4:(g4 + 1) * 4, :])
            wq = cst.tile([128, 2, C], bf16, tag="wq")
            nc.sync.dma_start(out=wq, in_=d_wq.ap())
            wkTb = cst.tile([128, 2, C], bf16, tag="wkTb")
            nc.scalar.dma_start(out=wkTb, in_=d_wkTb.ap())
            wvpf = cst.tile([128, 2, 2 * C], bf16, tag="wvpf")
            for cb in range(2):
                nc.scalar.dma_start(out=wvpf[:, :, cb * C:(cb + 1) * C],
                                    in_=d_wvpf.ap()[:, :, cb * C:(cb + 1) * C])
            wv88 = cst.tile([128, 2, 2 * C], fp8, tag="wv88")
            nc.scalar.dma_start(out=wv88, in_=d_wv88.ap())
            wv8 = wv88[:, :, 0:C]
            wp8 = wv88[:, :, C:2 * C]
            for q4 in range(2, 4):   # other key half, needed from pair 8
                for ci in range(2):
                    eng = nc.gpsimd if (ci == 0) else nc.sync
                    eng.dma_start(
                        out=x8[:, ci, q4 * 1024:(q4 + 1) * 1024],
                        in_=d_x8.ap()[:, ci, q4 * 1024:(q4 + 1) * 1024])
            xh = big.tile([128, 2, NH], bf16, tag="xh")
            for ci in range(2):
                for jh in range(2):
                    eng = nc.gpsimd if (jh == 0) else nc.sync
                    eng.dma_start(
                        out=xh[:, ci, jh * 1024:(jh + 1) * 1024],
                        in_=d_xh.ap()[:, ci, jh * 1024:(jh + 1) * 1024])

            # constants (DVE, tiny)
            epst = cst.tile([G, 1], f32, tag="epst")
            nc.vector.memset(epst, EPS)
            ones21t = cst.tile([128, 2, 16], fp8, tag="ones21")
            nc.vector.memset(ones21t, 1.0)
            ones21 = ones21t[:, :, 0:1]    # R lhsT (DR)
            negc = cst.tile([128, 1], f32, tag="negc")  # exp shift
            nc.vector.memset(negc, -3.0)
            r64s = cst.tile([1, 128], f32, tag="r64s")
            nc.vector.memset(r64s, 64.0)
            row64 = cst.tile([1, 128], f32r, tag="row64")   # 64/R bcast lhsT
            nc.vector.tensor_copy(out=row64, in_=r64s)
            inv64 = cst.tile([128, 1], f32, tag="inv64")
            nc.vector.memset(inv64, 1.0 / 64.0)

            # ---- GroupNorm sums via PE Gram on xT8 (DMA-paced; also the
            # PE warmup). G[:, ci, c'] = sum_m x[c,m] x[c',m]; col 128 =
            # sum_m x[c,m] (ones column).
            # two accumulators in DIFFERENT PSUM banks (an accumulation
            # group owns its 2KB zero region): ci0 in ps_t, ci1 borrows
            # ps_o (idle until chunk 0's po).
            gp0 = ps_t.tile([128, 132], f32, tag="t", name="gram0")
            gp1 = ps_o.tile([128, 132], f32, tag="o", name="gram1")
            gps = [gp0, gp1]
            for p in range(NP):
                for ci in range(2):
                    off = 0 if ci == 0 else C1OFF
                    nc.tensor.matmul(
                        gps[ci][:, 0:132],
                        lhsT=xT8[:, 2 * p:2 * p + 2, off:off + 128],
                        rhs=xT8[:, 2 * p:2 * p + 2, off:off + 132],
                        start=(p == 0), stop=(p == NP - 1), perf_mode=DR)

            # stats: st2[:, ci, 0] = sum x (ag carries 1/(CPG*HW)),
            # st2[:, ci, 1] = sum x^2 (diag of the Gram block).
            scr = gnp.tile([128, 2, 132], f32, tag="scr")
            st2 = gnp.tile([128, 2, 2], f32, tag="st2")
            bst = gnp.tile([128, 2, 6], f32, tag="bst")
            ba2 = gnp.tile([128, 2, 2], f32, tag="ba2")
            for ci in range(2):
                nc.vector.tensor_copy(out=st2[:, ci, 0:1],
                                      in_=gps[ci][:, 128:129])
                # diag extract: eye diagonal carries 129.0, so the mean
                # over the 129 columns of G*eye is exactly diag(G).
                nc.vector.tensor_tensor(
                    out=scr[:, ci, 0:129], in0=gps[ci][:, 0:129],
                    in1=eye[:, 0:129], op=OP.mult)
                nc.vector.bn_stats(out=bst[:, ci, :], in_=scr[:, ci, 0:129])
                nc.vector.bn_aggr(out=ba2[:, ci, :], in_=bst[:, ci, :])
                nc.vector.tensor_copy(out=st2[:, ci, 1:2],
                                      in_=ba2[:, ci, 0:1])
            pg = ps_t.tile([G, 2], f32, tag="t")
            for ci in range(2):
                nc.tensor.matmul(pg, lhsT=ag[:, ci, :], rhs=st2[:, ci, :],
                                 start=(ci == 0), stop=(ci == 1))
            pgs = gnp.tile([G, 2], f32, tag="pgs")
            nc.vector.tensor_copy(out=pgs, in_=pg)
            gst = gnp.tile([G, 4], f32, tag="gst")
            nc.vector.tensor_tensor(out=gst[:, 0:1], in0=pgs[:, 0:1],
                                    in1=pgs[:, 0:1], op=OP.mult)
            nc.vector.tensor_tensor(out=gst[:, 1:2], in0=pgs[:, 1:2],
                                    in1=gst[:, 0:1], op=OP.subtract)
            # rstd_g = exp(-0.5*ln(var+eps)): stays in the loaded tables
            gfin = gnp.tile([G, 2], f32, tag="gfin")  # (rstd_g, mean_g*rstd_g)
            # NOTE: Ln on ACT hard-crashes this HW (NRT_EXEC_UNIT_
            # UNRECOVERABLE) -- use Sqrt + DVE reciprocal. Costs one extra
            # act-table load (sqrt set vs exp set), both warmed early.
            nc.scalar.activation(out=gst[:, 2:3], in_=gst[:, 1:2],
                                 func=AF.Sqrt, bias=epst)
            # re-warm exp NOW: the exp-table reload (1.3us) runs while the
            # PE does pbc/qconv, instead of stalling the exp stream later
            nc.scalar.activation(out=warm[:, 0:1], in_=warm[:, 0:1],
                                 func=AF.Exp)
            nc.vector.reciprocal(out=gfin[:, 0:1], in_=gst[:, 2:3])
            nc.vector.tensor_tensor(out=gfin[:, 1:2], in0=pgs[:, 0:1],
                                    in1=gfin[:, 0:1], op=OP.mult)
            # bg carries gn_w: pbc = (scale_c, mean_c*scale_c);
            # bias_c = gn_b - mean_c*scale_c
            scbc = gnp.tile([128, 2, 2], f32, tag="scbc")
            for ci in range(2):
                pbc = ps_t.tile([128, 2], f32, tag="t")
                nc.tensor.matmul(pbc, lhsT=bg[:, ci, :], rhs=gfin,
                                 start=True, stop=True)
                nc.vector.tensor_copy(out=scbc[:, ci, 0:1], in_=pbc[:, 0:1])
                nc.vector.tensor_tensor(out=scbc[:, ci, 1:2], in0=gb[:, ci, :],
                                        in1=pbc[:, 1:2], op=OP.subtract)

            # q weights: fold GN scale, cast fp8 (q-conv is the only conv)
            w8q = cst.tile([128, 2, C], fp8, tag="w8q")
            for ci in range(2):
                nc.vector.tensor_scalar(
                    out=w8q[:, ci, :], in0=wq[:, ci, :],
                    scalar1=scbc[:, ci, 0:1], scalar2=None, op0=OP.mult)
            # per-partition scale for the ACT-side xe8 cast at boundaries
            sc64 = gnp.tile([128, 2, 1], f32, tag="sc64")
            nc.vector.tensor_scalar(
                out=sc64, in0=scbc[:, :, 0:1], scalar1=inv64, scalar2=None,
                op0=OP.mult)

            # bias chain:
            #   bias2q = qb + Wq^T bias_c         (per q out-channel)
            bcc = cst.tile([128, 2, 2], bf16, tag="bcc")
            for ci in range(2):
                nc.vector.tensor_copy(out=bcc[:, ci, 0:1], in_=scbc[:, ci, 1:2])
                nc.vector.tensor_copy(out=bcc[:, ci, 1:2], in_=scbc[:, ci, 1:2])
            bias2q = gnp.tile([128, 2, 1], f32, tag="bias2q")
            for cb in range(2):
                pbias = ps_t.tile([128, 2], f32, tag="t")
                for ci in range(2):
                    nc.tensor.matmul(
                        pbias, lhsT=wq[:, ci, cb * 128:(cb + 1) * 128],
                        rhs=bcc[:, ci, :], start=(ci == 0), stop=(ci == 1))
                nc.vector.tensor_tensor(
                    out=bias2q[:, cb, :], in0=pbias[:, 0:1],
                    in1=qb[:, cb, :], op=OP.add)

            # ---- q conv (fp8 DR) -> qt bf16; epilogues split DVE/ACT so
            # the t=0 pair finishes in one epilogue-latency. qt split by t
            # so chunk 0's qq gates on the two t=0 epilogues only.
            qts = [big.tile([128, 2, 2, 512], bf16, tag=f"qt{t}",
                            name=f"qt{t}") for t in range(2)]

            def q_epi(pq, cb, t):
                if cb == 0:
                    nc.vector.tensor_scalar(
                        out=qts[t][:, cb, :, :], in0=pq,
                        scalar1=bias2q[:, cb, :], scalar2=None, op0=OP.add)
                else:
                    nc.scalar.activation(
                        out=qts[t][:, cb, :, :], in_=pq,
                        func=AF.Identity, bias=bias2q[:, cb, :])

            qtiles = []
            for n in range(4):
                cb, t = n % 2, n // 2
                pq = ps_s.tile([128, 2, 512], f32, tag="s")
                for i in range(2):
                    j = 2 * t + i
                    nc.tensor.matmul(
                        pq[:, i, :], lhsT=w8q[:, :, cb * 128:(cb + 1) * 128],
                        rhs=x8[:, :, j * 512:(j + 1) * 512],
                        start=True, stop=True, perf_mode=DR)
                qtiles.append((pq, cb, t))
                if n >= 1:
                    q_epi(*qtiles[n - 1])
            q_epi(*qtiles[3])

            # ---- qq = s .* (Wk^T q): only chunk 0 up front; chunks 1..3
            # are produced inside the preceding chunk's pair loop (ps_t).
            qq8s = [big.tile([128, 2, 512], fp8, tag=f"qq8_{j}",
                             name=f"qq8_{j}") for j in range(4)]

            def qq_ci(j, ci):
                pqq = ps_t.tile([128, 512], f32, tag="t")
                for ch in range(2):
                    nc.tensor.matmul(
                        pqq, lhsT=wkTb[:, ch, ci * 128:(ci + 1) * 128],
                        rhs=qts[j // 2][:, ch, j % 2, :],
                        start=(ch == 0), stop=(ch == 1))
                nc.vector.tensor_scalar(
                    out=qq8s[j][:, ci, :], in0=pqq,
                    scalar1=scbc[:, ci, 0:1], scalar2=None, op0=OP.mult)

            qq_ci(0, 0)
            qq_ci(0, 1)

            # v/proj bias chain (tiny matmuls; nothing here gates the
            # attention start, so it is emitted after the warmup S-pairs):
            #   vb2 = Wv^T bias_c ; ub = Wp^T vb2 ; rbias2 = rbias + ub
            vb2pr = gnp.tile([128, 2, 2], bf16, tag="vb2pr")
            rbias2 = gnp.tile([128, 2, 1], f32, tag="rbias2")

            def emit_vbias():
                for cb in range(2):
                    pvb = ps_t.tile([128, 2], f32, tag="t")
                    for ci in range(2):
                        nc.tensor.matmul(
                            pvb, lhsT=wvpf[:, ci, cb * 128:(cb + 1) * 128],
                            rhs=bcc[:, ci, :], start=(ci == 0), stop=(ci == 1))
                    nc.vector.tensor_copy(out=vb2pr[:, cb, :], in_=pvb)
                for cb in range(2):
                    pub = ps_t.tile([128, 2], f32, tag="t")
                    for ch in range(2):
                        nc.tensor.matmul(
                            pub,
                            lhsT=wvpf[:, ch, C + cb * 128:C + (cb + 1) * 128],
                            rhs=vb2pr[:, ch, :], start=(ch == 0),
                            stop=(ch == 1))
                    nc.vector.tensor_tensor(
                        out=rbias2[:, cb, :], in0=pub[:, 0:1],
                        in1=rbias[:, cb, :], op=OP.add)

            # residual-with-bias tile; the adds are emitted inside chunk 0's
            # pair loop (DVE is idle there) -- needed first at tail(0)
            xo = big.tile([128, 2, NH], f32, tag="xo")

            def xo_step(n):
                cb, jj = n // 2, n % 2
                sl1 = slice(jj * 1024, (jj + 1) * 1024)
                nc.vector.tensor_scalar(
                    out=xo[:, cb, sl1], in0=xh[:, cb, sl1],
                    scalar1=rbias2[:, cb, :], scalar2=None, op0=OP.add)

            # ---- attention (pipelined within chunks AND across chunk
            # boundaries: the next chunk's first two S-pairs are emitted
            # before this chunk's tail so the exp stream never drains) ----
            NJ = NH // 512
            stiles = [[None] * NP for _ in range(NJ)]
            ets = [[None] * NP for _ in range(NJ)]

            def s_pair(j, g):
                st = ps_s.tile([128, 2, 512], f32, tag="s")
                stiles[j][g] = st
                for i in range(2):
                    mb = 2 * g + i
                    nc.tensor.matmul(
                        st[:, i, :],
                        lhsT=x8[:, :, mb * 128:(mb + 1) * 128],
                        rhs=qq8s[j], start=True, stop=True,
                        perf_mode=DR)

            def tail_head(po, on_act=False):
                """Free po fast: the two xe8 casts go one to DVE, one to
                ACT (idle at the boundary until the next chunk's first S
                lands), the reciprocal chain on DVE."""
                rinv = wrk.tile([1, 512], f32r, tag="rinv")
                xe8 = wrk.tile([128, 2, 512], fp8, tag="xe8")
                nc.vector.tensor_scalar(
                    out=xe8[:, 0, :], in0=po[:, 0, :],
                    scalar1=scbc[:, 0, 0:1], scalar2=inv64,
                    op0=OP.mult, op1=OP.mult)
                nc.scalar.activation(
                    out=xe8[:, 1, :], in_=po[:, 1, :],
                    func=AF.Identity, scale=sc64[:, 1, :])
                rinvf = wrk.tile([1, 512], f32, tag="rinvf")
                nc.vector.reciprocal_approx_fast(out=rinvf, in_=po[0:1, 2, :])
                if on_act:
                    nc.scalar.copy(out=rinv, in_=rinvf)
                else:
                    nc.vector.tensor_copy(out=rinv, in_=rinvf)
                return xe8, rinv

            def tail_steps(sl, xe8, rinv, po=None):
                """O = Wv^T xe8, proj, 64/R renorm, residual add, store.
                Deferred mode (po=None): all PSUM through ps_t (serial
                per-tile WAR chain), steps spread over the next chunk's
                pair loop so nothing blocks the in-order PE queue.
                Last-chunk mode (po given): use po's freed banks so the
                two Wv matmuls and the broadcast run without WAR stalls."""
                st = {}

                def wv(co):
                    def f():
                        if po is None:
                            pw = ps_t.tile([128, 512], f32, tag="t")
                        else:
                            pw = po[:, co, :]
                        st[("w", co)] = pw
                        nc.tensor.matmul(
                            pw, lhsT=wv8[:, :, co * 128:(co + 1) * 128],
                            rhs=xe8, start=True, stop=True, perf_mode=DR,
                            skip_group_check=True)
                    return f

                onorm = wrk.tile([128, 2, 512], fp8, tag="onorm")

                def onrm(co):
                    def f():
                        nc.vector.tensor_copy(out=onorm[:, co, :],
                                              in_=st[("w", co)])
                    return f

                def onrm_both():
                    nc.vector.tensor_copy(out=onorm, in_=po[:, 0:2, :])

                rb = wrk.tile([128, 512], f32, tag="rb")

                def bcast():
                    if po is None:
                        pbx = ps_t.tile([128, 512], f32, tag="t")
                    else:
                        pbx = po[:, 2, :]
                    nc.tensor.matmul(pbx, lhsT=row64, rhs=rinv,
                                     start=True, stop=True,
                                     skip_group_check=True)
                    nc.vector.tensor_copy(out=rb, in_=pbx)

                def proj(co, split=False):
                    def f():
                        if po is None or co == 0:
                            pp = ps_t.tile([128, 512], f32, tag="t",
                                           name="pp")
                        else:
                            pp = po[:, 1, :]
                        nc.tensor.matmul(
                            pp, lhsT=wp8[:, :, co * 128:(co + 1) * 128],
                            rhs=onorm, start=True, stop=True, perf_mode=DR,
                            skip_group_check=True)
                        outt = wrk.tile([128, 512], f32, tag="outt")
                        if not split:
                            nc.vector.tensor_tensor(out=outt, in0=pp, in1=rb,
                                                    op=OP.mult)
                            nc.vector.tensor_tensor(out=outt, in0=outt,
                                                    in1=xo[:, co, sl],
                                                    op=OP.add)
                            nc.sync.dma_start(out=d_out.ap()[:, co, sl],
                                              in_=outt)
                        else:
                            # last chunk: halves, stores on two rings so
                            # the final drain isn't one serial DMA chain
                            for h, eng in ((0, nc.sync), (1, nc.gpsimd)):
                                hs = slice(h * 256, (h + 1) * 256)
                                osl = slice(sl.start + h * 256,
                                            sl.start + (h + 1) * 256)
                                nc.vector.tensor_tensor(
                                    out=outt[:, hs], in0=pp[:, hs],
                                    in1=rb[:, hs], op=OP.mult)
                                nc.vector.tensor_tensor(
                                    out=outt[:, hs], in0=outt[:, hs],
                                    in1=xo[:, co, osl], op=OP.add)
                                eng.dma_start(out=d_out.ap()[:, co, osl],
                                              in_=outt[:, hs])
                    return f

                if po is None:
                    return [wv(0), wv(1), onrm(0), onrm(1), bcast,
                            proj(0), proj(1)]
                return [wv(0), wv(1), onrm_both, bcast,
                        proj(0, split=True), proj(1, split=True)]

            pending = []
            s_pair(0, 0)
            s_pair(0, 1)
            emit_vbias()
            for j in range(NJ):
                sl = slice(j * 512, (j + 1) * 512)
                po = ps_o.tile([128, 3, 512], f32, tag="o")  # xe c0, xe c1, R

                def xe_r(g, et, po=po):
                    for cb in range(2):
                        off = 0 if cb == 0 else C1OFF
                        nc.tensor.matmul(
                            po[:, cb, :],
                            lhsT=xT8[:, 2 * g:2 * g + 2, off:off + 128],
                            rhs=et, start=(g == 0), stop=(g == NP - 1),
                            perf_mode=DR, skip_group_check=True)
                    nc.tensor.matmul(
                        po[0:1, 2, :], lhsT=ones21, rhs=et,
                        start=(g == 0), stop=(g == NP - 1),
                        perf_mode=DR, skip_group_check=True)

                for g in range(NP):
                    et = epool.tile([128, 2, 512], fp8, tag="et")
                    ets[j][g] = et
                    nc.scalar.activation(out=et, in_=stiles[j][g], func=AF.Exp,
                                         scale=1.0 / 16.0, bias=negc)
                    # next chunk's qq early: fills the PE window while
                    # xe(0) still waits for the tail-head to release po
                    if j + 1 < NJ and g in (0, 1):
                        qq_ci(j + 1, g)
                    if g >= 1:
                        xe_r(g - 1, ets[j][g - 1])
                    if g + 2 <= NP - 1:
                        s_pair(j, g + 2)
                    # previous chunk's deferred tail, one step per iteration
                    # starting at g=2 (each step's deps are then ready and
                    # never stall the in-order PE queue)
                    if g >= 2 and pending:
                        pending.pop(0)()
                    # residual prep, spread over chunk 0 (DVE idle here)
                    if j == 0 and 2 <= g <= 5:
                        xo_step(g - 2)
                xe_r(NP - 1, ets[j][NP - 1])
                # next chunk's S warmup precedes this chunk's tail-head so
                # the exp stream never drains across the boundary
                if j + 1 < NJ:
                    xe8, rinv = tail_head(po)
                    s_pair(j + 1, 0)
                    s_pair(j + 1, 1)
                    pending = tail_steps(sl, xe8, rinv)
                else:
                    xe8, rinv = tail_head(po, on_act=True)
                    for f in tail_steps(sl, xe8, rinv, po=po):
                        f()

    nc.compile()
    _cache["nc"] = nc
    return nc


def _prep_maps(x, gn_w, gn_b, qkv_w, qkv_b, proj_w, proj_b):
    """Host-side sharding + layout prep. Returns list of 8 in_maps."""
    import ml_dtypes
    fp8 = ml_dtypes.float8_e4m3
    bf16 = ml_dtypes.bfloat16
    x = np.asarray(x, np.float32)
    qkv_w = np.asarray(qkv_w, np.float32)
    qkv_b = np.asarray(qkv_b, np.float32)
    proj_w = np.asarray(proj_w, np.float32)
    proj_b = np.asarray(proj_b, np.float32)
    gn_w = np.asarray(gn_w, np.float32)
    gn_b = np.asarray(gn_b, np.float32)

    def chunked(a):  # [256, ...] -> [128, 2, ...]
        return np.ascontiguousarray(a.reshape(2, 128, *a.shape[1:]).transpose(
            1, 0, *range(2, a.ndim + 1)))

    wq = chunked(qkv_w[0:C].T.copy()).astype(bf16)       # [c_in, 2, c_out]
    wvf = chunked(qkv_w[2 * C:3 * C].T.copy())           # [c_in, 2, c_out]
    wpf = chunked(proj_w.T.copy())                       # [c_in, 2, c_out]
    wvpf = np.concatenate([wvf, wpf], axis=2).astype(bf16)
    wkTb = chunked(qkv_w[C:2 * C].copy()).astype(bf16)   # [c_out, 2, c_in]
    wv88 = np.concatenate([wvf, wpf], axis=2).astype(fp8)
    rbias = proj_w @ qkv_b[2 * C:3 * C] + proj_b   # v-bias fold + proj bias
    kb_unused = np.zeros(C, np.float32)
    smalls = np.stack([qkv_b[0:C], kb_unused, gn_w, gn_b, rbias], axis=1)

    cidx = np.arange(C)
    ag_full = (cidx[:, None] // CPG == np.arange(G)[None, :]).astype(np.float32)
    ag = ag_full / (CPG * HW)                       # carries 1/(8*4096)
    hdr = chunked(np.concatenate([smalls, ag], axis=1))  # [128, 2, 37]
    bg_full = ag_full * gn_w[:, None]               # carries gn_w
    bg = np.ascontiguousarray(
        bg_full.reshape(2, 128, G).transpose(2, 0, 1))  # [G, 2, 128]
    eye = np.zeros((128, 132), np.float32)
    eye[np.arange(128), np.arange(128)] = 129.0
    eye = eye.astype(bf16)

    maps = []
    for core in range(8):
        b, half = core // 2, core % 2
        xf = x[b].reshape(C, HW)
        if half == 1:   # own query half first (key order is irrelevant
            xf = np.concatenate([xf[:, NH:], xf[:, :NH]], axis=1)
        xf = np.ascontiguousarray(xf)
        xhm = xf[:, 0:NH]                            # own half = residual
        x8c = chunked(xf).astype(fp8)
        # xT8e: [pix128, MB, XW]: [x_c0 | ones | pad | x_c1 | ones | pad]
        xT = np.ascontiguousarray(
            xf.T.reshape(MB, 128, C).transpose(1, 0, 2)).astype(fp8)
        xT8e = np.zeros((128, MB, XW), fp8)
        xT8e[:, :, 0:128] = xT[:, :, 0:128]
        xT8e[:, :, 128] = np.float32(1.0)
        xT8e[:, :, C1OFF:C1OFF + 128] = xT[:, :, 128:256]
        xT8e[:, :, C1OFF + 128] = np.float32(1.0)
        maps.append({
            "x8": x8c,
            "xT8": xT8e, "xh": chunked(xhm).astype(bf16),
            "wq": wq, "wvpf": wvpf,
            "wkTb": wkTb, "wv88": wv88,
            "hdr": hdr, "bg": bg, "eye": eye,
        })
    return maps


def kernel(x, gn_w, gn_b, qkv_w, qkv_b, proj_w, proj_b):
    import concourse.bass_utils as bu
    nc = build_nc()
    maps = _prep_maps(x, gn_w, gn_b, qkv_w, qkv_b, proj_w, proj_b)
    res = bu.run_bass_kernel_spmd(nc, maps, core_ids=list(range(8)))
    out = np.empty((B, C, HW), np.float32)
    for core in range(8):
        b, half = core // 2, core % 2
        o = res.results[core]["out"]                # [128, 2, NH]
        out[b, :, half * NH:(half + 1) * NH] = \
            o.transpose(1, 0, 2).reshape(C, NH)
    return out.reshape(B, C, 64, 64)


# revision 15
# speedup vs baseline: 1.0526x; 1.0129x over previous
"""AttentionBlock (B=4, C=256, H=W=64) on 8 Trainium2 NeuronCores.

Sharding: data-parallel over (batch, query-half): core i handles batch i//2,
query pixels [half*2048, (half+1)*2048), half = i%2. GroupNorm stats are
computed per batch element (duplicated across the pair, cheap); the O(N^2)
attention work is fully sharded 8 ways. No collectives.

v5: v4's fp8 DoubleRow attention core (already at the fp8 PE roofline in
steady state) with the serial head rebuilt around the PE:

  - GroupNorm sums come from PE Gram matmuls on xT8e (x^T x per channel
    block with an appended ones column -> sum x and sum x^2 per channel),
    accumulating while the DMA load streams in. This removes the 16-op
    bn_stats chain from DVE and doubles as the PE p-state warmup.
    Diagonal extraction via tensor_tensor_reduce against a host eye mask;
    rstd_g = exp(-0.5*ln(var+eps)) keeps ACT inside one table family
    (ln/exp/identity/copy) so the 1.3us table loads happen once, early.
  - x8h is gone: per-core host prep permutes key blocks so the core's own
    query half occupies x8 columns [0, 2048) -- the q-conv reads x8
    directly and the S/xe key order (attention is key-order invariant)
    follows the same permutation in xT8e.
  - xh (residual) ships as bf16, wq/wvf/wpf as bf16: less ring time, no
    meaningful precision change (biases are small; w8q is fp8 anyway).
  - DMA as few large transfers spread over the 4 DGE rings (~600ns of
    engine queue time each), ordered by criticality; ACT ring stays empty
    so the activation warms run immediately.
  - q-conv epilogues split ACT/DVE; vbias fills the qq-cast gap; at chunk
    boundaries ACT takes one of the two xe8 casts so the next chunk's
    xe(0) sees po freed ~0.6us earlier; last chunk stores split over two
    rings.

Attention core (unchanged from v4):
  S[m,n]  = sum_c x[c,m] * qq[c,n],   qq = s .* (Wk^T q)   (k-conv fused)
  O[o,n]  = sum_c Wv[c,o] * s[c] * xe[c,n],  xe = sum_m x[c,m] E[m,n]
  E = exp(S/16 - 3.0) in fp8 (shift cancels via R; -3.0 keeps the
  data's max S ~138 clear of fp8e4's 448 ceiling); xe accumulated /64-scaled in fp8, the x64
  restored via the R-broadcast; R (softmax denominator) via a DoubleRow
  ones-matmul accumulated alongside xe; 1/R via the fast DVE reciprocal.
  Per 512-query chunk: 16 key-block pairs, software-pipelined so the PE
  executes {S(g+1), xe(g-1), R(g-1)} during exp(g)."""

import numpy as np

B, C, HW = 4, 256, 4096
NH = 2048            # query pixels per core
G, CPG = 32, 8       # groups, channels per group
EPS = 1e-5
MB = HW // 128       # 32 key blocks
NP = MB // 2         # 16 key-block pairs
XW = 288             # xT8e padded row: [x_c0 0:128 | ones 128 | x_c1 144:272 | ones 272]
C1OFF = 144

_cache = {}


def build_nc():
    """Build (and cache) the Bass module."""
    if "nc" in _cache:
        return _cache["nc"]
    import concourse.tile as tile
    from concourse import bacc, mybir

    f32 = mybir.dt.float32
    f32r = mybir.dt.float32r
    fp8 = mybir.dt.float8e4
    bf16 = mybir.dt.bfloat16
    AF = mybir.ActivationFunctionType
    OP = mybir.AluOpType
    DR = mybir.MatmulPerfMode.DoubleRow

    nc = bacc.Bacc("TRN2", target_bir_lowering=False, debug=False,
                   enable_asserts=False, num_devices=8)

    # ---- DRAM I/O (host preps everything into device layout) ----
    d_x8 = nc.dram_tensor("x8", [128, 2, HW], fp8, kind="ExternalInput")
    d_xT8 = nc.dram_tensor("xT8", [128, MB, XW], fp8, kind="ExternalInput")
    d_xh = nc.dram_tensor("xh", [128, 2, NH], bf16, kind="ExternalInput")
    d_wq = nc.dram_tensor("wq", [128, 2, C], bf16, kind="ExternalInput")
    d_wvpf = nc.dram_tensor("wvpf", [128, 2, 2 * C], bf16, kind="ExternalInput")
    d_wkTb = nc.dram_tensor("wkTb", [128, 2, C], bf16, kind="ExternalInput")
    d_wv88 = nc.dram_tensor("wv88", [128, 2, 2 * C], fp8, kind="ExternalInput")
    d_hdr = nc.dram_tensor("hdr", [128, 2, 5 + G], f32, kind="ExternalInput")
    d_bg = nc.dram_tensor("bg", [G, 2, 128], f32, kind="ExternalInput")
    d_eye = nc.dram_tensor("eye", [128, 132], bf16, kind="ExternalInput")
    d_out = nc.dram_tensor("out", [128, 2, NH], f32, kind="ExternalOutput")

    with tile.TileContext(nc) as tc:
        with (
            tc.tile_pool(name="big", bufs=1) as big,
            tc.tile_pool(name="cst", bufs=1) as cst,
            tc.tile_pool(name="wrk", bufs=3) as wrk,
            tc.tile_pool(name="epool", bufs=6) as epool,
            tc.tile_pool(name="gnp", bufs=1) as gnp,
            tc.tile_pool(name="ps_s", bufs=2, space="PSUM") as ps_s,
            tc.tile_pool(name="ps_o", bufs=1, space="PSUM") as ps_o,
            tc.tile_pool(name="ps_t", bufs=1, space="PSUM") as ps_t,
        ):
            # ---- ACT warm: exp then ln on a scratch tile. The act-table
            # pass inserts the (up to two) table loads right here, ~7us,
            # while ACT is otherwise idle; every later activation (ln,
            # exp, identity, copy) hits a loaded table.
            warm = cst.tile([1, 2], f32, tag="warm")
            nc.vector.memset(warm, 1.0)
            nc.scalar.activation(out=warm[:, 0:1], in_=warm[:, 0:1],
                                 func=AF.Exp)

            # ---- input loads. Per-dispatch engine-queue cost is ~600ns,
            # so: few, large transfers over the 3 DGE rings (SP/ACT/Pool),
            # ordered by criticality: xT8e (stats) on sync+gpsimd, own-half
            # x8 (q-conv) early on the ACT ring, trailing bulk last.
            hdr = cst.tile([128, 2, 5 + G], f32, tag="hdr")
            bg = cst.tile([G, 2, 128], f32, tag="bg")
            eye = cst.tile([128, 132], bf16, tag="eye")
            qb = hdr[:, :, 0:1]
            gb = hdr[:, :, 3:4]
            rbias = hdr[:, :, 4:5]
            ag = hdr[:, :, 5:5 + G]

            # x8: own query half (cols 0:2048 after the host block
            # permutation) first -- feeds q-conv; S pairs then consume
            # blocks in permuted order, covered by the halves.
            # NOTE: one transfer rides ONE DMA engine (~22.5 GB/s), so
            # bandwidth needs MANY in-flight transfers: ~64-148KB slices,
            # dispatched round-robin (each costs ~600ns of engine queue).
            x8 = big.tile([128, 2, HW], fp8, tag="x8")
            for q4 in range(2):      # own query half: q-conv + S pairs 0-7
                for ci in range(2):
                    nc.scalar.dma_start(
                        out=x8[:, ci, q4 * 1024:(q4 + 1) * 1024],
                        in_=d_x8.ap()[:, ci, q4 * 1024:(q4 + 1) * 1024])
            xT8 = big.tile([128, MB, XW], fp8, tag="xT8")
            for g4 in range(8):      # 4-block groups (148KB), 2 rings,
                eng = nc.gpsimd if (g4 % 2 == 0) else nc.sync
                eng.dma_start(out=xT8[:, g4 * 4:(g4 + 1) * 4, :],
                              in_=d_xT8.ap()[:, g4 * 4:(g4 + 1) * 4, :])
            nc.gpsimd.dma_start(out=hdr, in_=d_hdr.ap())
            nc.gpsimd.dma_start(out=bg, in_=d_bg.ap())
            nc.gpsimd.dma_start(out=eye, in_=d_eye.ap())
            wq = cst.tile([128, 2, C], bf16, tag="wq")
            nc.scalar.dma_start(out=wq, in_=d_wq.ap())
            wkTb = cst.tile([128, 2, C], bf16, tag="wkTb")
            nc.scalar.dma_start(out=wkTb, in_=d_wkTb.ap())
            wvpf = cst.tile([128, 2, 2 * C], bf16, tag="wvpf")
            for cb in range(2):
                nc.scalar.dma_start(out=wvpf[:, :, cb * C:(cb + 1) * C],
                                    in_=d_wvpf.ap()[:, :, cb * C:(cb + 1) * C])
            wv88 = cst.tile([128, 2, 2 * C], fp8, tag="wv88")
            nc.scalar.dma_start(out=wv88, in_=d_wv88.ap())
            wv8 = wv88[:, :, 0:C]
            wp8 = wv88[:, :, C:2 * C]
            for q4 in range(2, 4):   # other key half, needed from pair 8
                for ci in range(2):
                    eng = nc.gpsimd if (ci == 0) else nc.sync
                    eng.dma_start(
                        out=x8[:, ci, q4 * 1024:(q4 + 1) * 1024],
                        in_=d_x8.ap()[:, ci, q4 * 1024:(q4 + 1) * 1024])
            xh = big.tile([128, 2, NH], bf16, tag="xh")
            for ci in range(2):
                for jh in range(2):
                    eng = nc.gpsimd if (jh == 0) else nc.sync
                    eng.dma_start(
                        out=xh[:, ci, jh * 1024:(jh + 1) * 1024],
                        in_=d_xh.ap()[:, ci, jh * 1024:(jh + 1) * 1024])

            # constants (DVE, tiny)
            epst = cst.tile([G, 1], f32, tag="epst")
            nc.vector.memset(epst, EPS)
            ones21t = cst.tile([128, 2, 16], fp8, tag="ones21")
            nc.vector.memset(ones21t, 1.0)
            ones21 = ones21t[:, :, 0:1]    # R lhsT (DR)
            negc = cst.tile([128, 1], f32, tag="negc")  # exp shift
            nc.vector.memset(negc, -3.0)
            r64s = cst.tile([1, 128], f32, tag="r64s")
            nc.vector.memset(r64s, 64.0)
            row64 = cst.tile([1, 128], f32r, tag="row64")   # 64/R bcast lhsT
            nc.vector.tensor_copy(out=row64, in_=r64s)
            inv64 = cst.tile([128, 1], f32, tag="inv64")
            nc.vector.memset(inv64, 1.0 / 64.0)

            # ---- GroupNorm sums via PE Gram on xT8 (DMA-paced; also the
            # PE warmup). G[:, ci, c'] = sum_m x[c,m] x[c',m]; col 128 =
            # sum_m x[c,m] (ones column).
            # two accumulators in DIFFERENT PSUM banks (an accumulation
            # group owns its 2KB zero region): ci0 in ps_t, ci1 borrows
            # ps_o (idle until chunk 0's po).
            gp0 = ps_t.tile([128, 132], f32, tag="t", name="gram0")
            gp1 = ps_o.tile([128, 132], f32, tag="o", name="gram1")
            gps = [gp0, gp1]
            for p in range(NP):
                for ci in range(2):
                    off = 0 if ci == 0 else C1OFF
                    nc.tensor.matmul(
                        gps[ci][:, 0:132],
                        lhsT=xT8[:, 2 * p:2 * p + 2, off:off + 128],
                        rhs=xT8[:, 2 * p:2 * p + 2, off:off + 132],
                        start=(p == 0), stop=(p == NP - 1), perf_mode=DR)

            # stats: st2[:, ci, 0] = sum x (ag carries 1/(CPG*HW)),
            # st2[:, ci, 1] = sum x^2 (diag of the Gram block).
            scr = gnp.tile([128, 2, 132], f32, tag="scr")
            st2 = gnp.tile([128, 2, 2], f32, tag="st2")
            bst = gnp.tile([128, 2, 6], f32, tag="bst")
            ba2 = gnp.tile([128, 2, 2], f32, tag="ba2")
            for ci in range(2):
                nc.vector.tensor_copy(out=st2[:, ci, 0:1],
                                      in_=gps[ci][:, 128:129])
                # diag extract: eye diagonal carries 129.0, so the mean
                # over the 129 columns of G*eye is exactly diag(G).
                nc.vector.tensor_tensor(
                    out=scr[:, ci, 0:129], in0=gps[ci][:, 0:129],
                    in1=eye[:, 0:129], op=OP.mult)
                nc.vector.bn_stats(out=bst[:, ci, :], in_=scr[:, ci, 0:129])
                nc.vector.bn_aggr(out=ba2[:, ci, :], in_=bst[:, ci, :])
                nc.vector.tensor_copy(out=st2[:, ci, 1:2],
                                      in_=ba2[:, ci, 0:1])
            pg = ps_t.tile([G, 2], f32, tag="t")
            for ci in range(2):
                nc.tensor.matmul(pg, lhsT=ag[:, ci, :], rhs=st2[:, ci, :],
                                 start=(ci == 0), stop=(ci == 1))
            pgs = gnp.tile([G, 2], f32, tag="pgs")
            nc.vector.tensor_copy(out=pgs, in_=pg)
            gst = gnp.tile([G, 4], f32, tag="gst")
            nc.vector.tensor_tensor(out=gst[:, 0:1], in0=pgs[:, 0:1],
                                    in1=pgs[:, 0:1], op=OP.mult)
            nc.vector.tensor_tensor(out=gst[:, 1:2], in0=pgs[:, 1:2],
                                    in1=gst[:, 0:1], op=OP.subtract)
            # rstd_g = exp(-0.5*ln(var+eps)): stays in the loaded tables
            gfin = gnp.tile([G, 2], f32, tag="gfin")  # (rstd_g, mean_g*rstd_g)
            # rstd_g on DVE only (ACT Ln crashes this HW; ACT Sqrt costs
            # act-table swaps that stall the exp stream): Quake-style
            # rsqrt seed + 2 Newton steps, all [G,1] ops, ~5e-6 rel.
            i32 = mybir.dt.int32
            nw = gnp.tile([G, 4], f32, tag="nw")   # v+eps, -0.5v, y, scratch
            nc.vector.tensor_scalar(out=nw[:, 0:1], in0=gst[:, 1:2],
                                    scalar1=EPS, scalar2=None, op0=OP.add)
            nc.vector.tensor_scalar(out=nw[:, 1:2], in0=nw[:, 0:1],
                                    scalar1=-0.5, scalar2=None, op0=OP.mult)
            nc.vector.tensor_scalar(
                out=nw[:, 2:3].bitcast(i32), in0=nw[:, 0:1].bitcast(i32),
                scalar1=1, scalar2=None, op0=OP.logical_shift_right)
            nc.vector.tensor_scalar(
                out=nw[:, 2:3].bitcast(i32), in0=nw[:, 2:3].bitcast(i32),
                scalar1=-1, scalar2=0x5f3759df, op0=OP.mult, op1=OP.add)
            for _ in range(2):
                nc.vector.tensor_tensor(out=nw[:, 3:4], in0=nw[:, 2:3],
                                        in1=nw[:, 2:3], op=OP.mult)
                nc.vector.tensor_tensor(out=nw[:, 3:4], in0=nw[:, 3:4],
                                        in1=nw[:, 1:2], op=OP.mult)
                nc.vector.tensor_scalar(out=nw[:, 3:4], in0=nw[:, 3:4],
                                        scalar1=1.5, scalar2=None, op0=OP.add)
                nc.vector.tensor_tensor(out=nw[:, 2:3], in0=nw[:, 2:3],
                                        in1=nw[:, 3:4], op=OP.mult)
            nc.vector.tensor_copy(out=gfin[:, 0:1], in_=nw[:, 2:3])
            nc.vector.tensor_tensor(out=gfin[:, 1:2], in0=pgs[:, 0:1],
                                    in1=gfin[:, 0:1], op=OP.mult)
            # bg carries gn_w: pbc = (scale_c, mean_c*scale_c);
            # bias_c = gn_b - mean_c*scale_c
            scbc = gnp.tile([128, 2, 2], f32, tag="scbc")
            for ci in range(2):
                pbc = ps_t.tile([128, 2], f32, tag="t")
                nc.tensor.matmul(pbc, lhsT=bg[:, ci, :], rhs=gfin,
                                 start=True, stop=True)
                nc.vector.tensor_copy(out=scbc[:, ci, 0:1], in_=pbc[:, 0:1])
                nc.vector.tensor_tensor(out=scbc[:, ci, 1:2], in0=gb[:, ci, :],
                                        in1=pbc[:, 1:2], op=OP.subtract)

            # q weights: fold GN scale, cast fp8 (q-conv is the only conv)
            w8q = cst.tile([128, 2, C], fp8, tag="w8q")
            for ci in range(2):
                nc.vector.tensor_scalar(
                    out=w8q[:, ci, :], in0=wq[:, ci, :],
                    scalar1=scbc[:, ci, 0:1], scalar2=None, op0=OP.mult)
            # bias chain:
            #   bias2q = qb + Wq^T bias_c         (per q out-channel)
            bcc = cst.tile([128, 2, 2], bf16, tag="bcc")
            for ci in range(2):
                nc.vector.tensor_copy(out=bcc[:, ci, 0:1], in_=scbc[:, ci, 1:2])
                nc.vector.tensor_copy(out=bcc[:, ci, 1:2], in_=scbc[:, ci, 1:2])
            # per-partition scale for the ACT-side xe8 cast at boundaries
            sc64 = gnp.tile([128, 2, 1], f32, tag="sc64")
            nc.vector.tensor_scalar(
                out=sc64, in0=scbc[:, :, 0:1], scalar1=inv64, scalar2=None,
                op0=OP.mult)
            bias2q = gnp.tile([128, 2, 1], f32, tag="bias2q")
            for cb in range(2):
                pbias = ps_t.tile([128, 2], f32, tag="t")
                for ci in range(2):
                    nc.tensor.matmul(
                        pbias, lhsT=wq[:, ci, cb * 128:(cb + 1) * 128],
                        rhs=bcc[:, ci, :], start=(ci == 0), stop=(ci == 1))
                nc.vector.tensor_tensor(
                    out=bias2q[:, cb, :], in0=pbias[:, 0:1],
                    in1=qb[:, cb, :], op=OP.add)

            # ---- q conv (fp8 DR) -> qt bf16; epilogues split DVE/ACT so
            # the t=0 pair finishes in one epilogue-latency. qt split by t
            # so chunk 0's qq gates on the two t=0 epilogues only.
            qts = [big.tile([128, 2, 2, 512], bf16, tag=f"qt{t}",
                            name=f"qt{t}") for t in range(2)]

            def q_epi(pq, cb, t):
                if cb == 0:
                    nc.vector.tensor_scalar(
                        out=qts[t][:, cb, :, :], in0=pq,
                        scalar1=bias2q[:, cb, :], scalar2=None, op0=OP.add)
                else:
                    nc.scalar.activation(
                        out=qts[t][:, cb, :, :], in_=pq,
                        func=AF.Identity, bias=bias2q[:, cb, :])

            qtiles = []
            for n in range(4):
                cb, t = n % 2, n // 2
                pq = ps_s.tile([128, 2, 512], f32, tag="s")
                for i in range(2):
                    j = 2 * t + i
                    nc.tensor.matmul(
                        pq[:, i, :], lhsT=w8q[:, :, cb * 128:(cb + 1) * 128],
                        rhs=x8[:, :, j * 512:(j + 1) * 512],
                        start=True, stop=True, perf_mode=DR)
                qtiles.append((pq, cb, t))
                if n >= 1:
                    q_epi(*qtiles[n - 1])
            q_epi(*qtiles[3])

            # ---- qq = s .* (Wk^T q): only chunk 0 up front; chunks 1..3
            # are produced inside the preceding chunk's pair loop (ps_t).
            qq8s = [big.tile([128, 2, 512], fp8, tag=f"qq8_{j}",
                             name=f"qq8_{j}") for j in range(4)]

            def qq_ci(j, ci):
                pqq = ps_t.tile([128, 512], f32, tag="t")
                for ch in range(2):
                    nc.tensor.matmul(
                        pqq, lhsT=wkTb[:, ch, ci * 128:(ci + 1) * 128],
                        rhs=qts[j // 2][:, ch, j % 2, :],
                        start=(ch == 0), stop=(ch == 1))
                nc.vector.tensor_scalar(
                    out=qq8s[j][:, ci, :], in0=pqq,
                    scalar1=scbc[:, ci, 0:1], scalar2=None, op0=OP.mult)

            qq_ci(0, 0)
            qq_ci(0, 1)

            # v/proj bias chain (tiny matmuls; nothing here gates the
            # attention start, so it is emitted after the warmup S-pairs):
            #   vb2 = Wv^T bias_c ; ub = Wp^T vb2 ; rbias2 = rbias + ub
            vb2pr = gnp.tile([128, 2, 2], bf16, tag="vb2pr")
            rbias2 = gnp.tile([128, 2, 1], f32, tag="rbias2")

            def emit_vbias():
                for cb in range(2):
                    pvb = ps_t.tile([128, 2], f32, tag="t")
                    for ci in range(2):
                        nc.tensor.matmul(
                            pvb, lhsT=wvpf[:, ci, cb * 128:(cb + 1) * 128],
                            rhs=bcc[:, ci, :], start=(ci == 0), stop=(ci == 1))
                    nc.vector.tensor_copy(out=vb2pr[:, cb, :], in_=pvb)
                for cb in range(2):
                    pub = ps_t.tile([128, 2], f32, tag="t")
                    for ch in range(2):
                        nc.tensor.matmul(
                            pub,
                            lhsT=wvpf[:, ch, C + cb * 128:C + (cb + 1) * 128],
                            rhs=vb2pr[:, ch, :], start=(ch == 0),
                            stop=(ch == 1))
                    nc.vector.tensor_tensor(
                        out=rbias2[:, cb, :], in0=pub[:, 0:1],
                        in1=rbias[:, cb, :], op=OP.add)

            # residual-with-bias tile; the adds are emitted inside chunk 0's
            # pair loop (DVE is idle there) -- needed first at tail(0)
            xo = big.tile([128, 2, NH], f32, tag="xo")

            def xo_step(n):
                cb, jj = n // 2, n % 2
                sl1 = slice(jj * 1024, (jj + 1) * 1024)
                nc.vector.tensor_scalar(
                    out=xo[:, cb, sl1], in0=xh[:, cb, sl1],
                    scalar1=rbias2[:, cb, :], scalar2=None, op0=OP.add)

            # ---- attention (pipelined within chunks AND across chunk
            # boundaries: the next chunk's first two S-pairs are emitted
            # before this chunk's tail so the exp stream never drains) ----
            NJ = NH // 512
            stiles = [[None] * NP for _ in range(NJ)]
            ets = [[None] * NP for _ in range(NJ)]

            def s_pair(j, g):
                st = ps_s.tile([128, 2, 512], f32, tag="s")
                stiles[j][g] = st
                for i in range(2):
                    mb = 2 * g + i
                    nc.tensor.matmul(
                        st[:, i, :],
                        lhsT=x8[:, :, mb * 128:(mb + 1) * 128],
                        rhs=qq8s[j], start=True, stop=True,
                        perf_mode=DR)

            def tail_head(po, on_act=False):
                """Free po fast: the two xe8 casts go one to DVE, one to
                ACT (idle at the boundary until the next chunk's first S
                lands), the reciprocal chain on DVE."""
                rinv = wrk.tile([1, 512], f32r, tag="rinv")
                xe8 = wrk.tile([128, 2, 512], fp8, tag="xe8")
                nc.vector.tensor_scalar(
                    out=xe8[:, 0, :], in0=po[:, 0, :],
                    scalar1=scbc[:, 0, 0:1], scalar2=inv64,
                    op0=OP.mult, op1=OP.mult)
                nc.scalar.activation(
                    out=xe8[:, 1, :], in_=po[:, 1, :],
                    func=AF.Identity, scale=sc64[:, 1, :])
                rinvf = wrk.tile([1, 512], f32, tag="rinvf")
                nc.vector.reciprocal_approx_fast(out=rinvf, in_=po[0:1, 2, :])
                if on_act:
                    nc.scalar.copy(out=rinv, in_=rinvf)
                else:
                    nc.vector.tensor_copy(out=rinv, in_=rinvf)
                return xe8, rinv

            def tail_steps(sl, xe8, rinv, po=None):
                """O = Wv^T xe8, proj, 64/R renorm, residual add, store.
                Deferred mode (po=None): all PSUM through ps_t (serial
                per-tile WAR chain), steps spread over the next chunk's
                pair loop so nothing blocks the in-order PE queue.
                Last-chunk mode (po given): use po's freed banks so the
                two Wv matmuls and the broadcast run without WAR stalls."""
                st = {}

                def wv(co):
                    def f():
                        if po is None:
                            pw = ps_t.tile([128, 512], f32, tag="t")
                        else:
                            pw = po[:, co, :]
                        st[("w", co)] = pw
                        nc.tensor.matmul(
                            pw, lhsT=wv8[:, :, co * 128:(co + 1) * 128],
                            rhs=xe8, start=True, stop=True, perf_mode=DR,
                            skip_group_check=True)
                    return f

                onorm = wrk.tile([128, 2, 512], fp8, tag="onorm")

                def onrm(co):
                    def f():
                        nc.vector.tensor_copy(out=onorm[:, co, :],
                                              in_=st[("w", co)])
                    return f

                def onrm_both():
                    nc.vector.tensor_copy(out=onorm, in_=po[:, 0:2, :])

                rb = wrk.tile([128, 512], f32, tag="rb")

                def bcast():
                    if po is None:
                        pbx = ps_t.tile([128, 512], f32, tag="t")
                    else:
                        pbx = po[:, 2, :]
                    nc.tensor.matmul(pbx, lhsT=row64, rhs=rinv,
                                     start=True, stop=True,
                                     skip_group_check=True)
                    nc.vector.tensor_copy(out=rb, in_=pbx)

                def proj(co, split=False):
                    def f():
                        if po is None or co == 0:
                            pp = ps_t.tile([128, 512], f32, tag="t",
                                           name="pp")
                        else:
                            pp = po[:, 1, :]
                        nc.tensor.matmul(
                            pp, lhsT=wp8[:, :, co * 128:(co + 1) * 128],
                            rhs=onorm, start=True, stop=True, perf_mode=DR,
                            skip_group_check=True)
                        outt = wrk.tile([128, 512], f32, tag="outt")
                        if not split:
                            nc.vector.tensor_tensor(out=outt, in0=pp, in1=rb,
                                                    op=OP.mult)
                            nc.vector.tensor_tensor(out=outt, in0=outt,
                                                    in1=xo[:, co, sl],
                                                    op=OP.add)
                            nc.sync.dma_start(out=d_out.ap()[:, co, sl],
                                              in_=outt)
                        else:
                            # last chunk: halves, stores on two rings so
                            # the final drain isn't one serial DMA chain
                            for h, eng in ((0, nc.sync), (1, nc.gpsimd)):
                                hs = slice(h * 256, (h + 1) * 256)
                                osl = slice(sl.start + h * 256,
                                            sl.start + (h + 1) * 256)
                                nc.vector.tensor_tensor(
                                    out=outt[:, hs], in0=pp[:, hs],
                                    in1=rb[:, hs], op=OP.mult)
                                nc.vector.tensor_tensor(
                                    out=outt[:, hs], in0=outt[:, hs],
                                    in1=xo[:, co, osl], op=OP.add)
                                eng.dma_start(out=d_out.ap()[:, co, osl],
                                              in_=outt[:, hs])
                    return f

                if po is None:
                    return [wv(0), wv(1), onrm(0), onrm(1), bcast,
                            proj(0), proj(1)]
                return [wv(0), wv(1), onrm_both, bcast,
                        proj(0, split=True), proj(1, split=True)]

            pending = []
            s_pair(0, 0)
            s_pair(0, 1)
            emit_vbias()
            for j in range(NJ):
                sl = slice(j * 512, (j + 1) * 512)
                po = ps_o.tile([128, 3, 512], f32, tag="o")  # xe c0, xe c1, R

                def xe_r(g, et, po=po):
                    for cb in range(2):
                        off = 0 if cb == 0 else C1OFF
                        nc.tensor.matmul(
                            po[:, cb, :],
                            lhsT=xT8[:, 2 * g:2 * g + 2, off:off + 128],
                            rhs=et, start=(g == 0), stop=(g == NP - 1),
                            perf_mode=DR, skip_group_check=True)
                    nc.tensor.matmul(
                        po[0:1, 2, :], lhsT=ones21, rhs=et,
                        start=(g == 0), stop=(g == NP - 1),
                        perf_mode=DR, skip_group_check=True)

                for g in range(NP):
                    et = epool.tile([128, 2, 512], fp8, tag="et")
                    ets[j][g] = et
                    nc.scalar.activation(out=et, in_=stiles[j][g], func=AF.Exp,
                                         scale=1.0 / 16.0, bias=negc)
                    # next chunk's qq early: fills the PE window while
                    # xe(0) still waits for the tail-head to release po
                    if j + 1 < NJ and g in (0, 1):
                        qq_ci(j + 1, g)
                    if g >= 1:
                        xe_r(g - 1, ets[j][g - 1])
                    if g + 2 <= NP - 1:
                        s_pair(j, g + 2)
                    # previous chunk's deferred tail, one step per iteration
                    # starting at g=2 (each step's deps are then ready and
                    # never stall the in-order PE queue)
                    if g >= 2 and pending:
                        pending.pop(0)()
                    # residual prep, spread over chunk 0 (DVE idle here)
                    if j == 0 and 2 <= g <= 5:
                        xo_step(g - 2)
                xe_r(NP - 1, ets[j][NP - 1])
                # next chunk's S warmup precedes this chunk's tail-head so
                # the exp stream never drains across the boundary
                if j + 1 < NJ:
                    xe8, rinv = tail_head(po)
                    s_pair(j + 1, 0)
                    s_pair(j + 1, 1)
                    pending = tail_steps(sl, xe8, rinv)
                else:
                    xe8, rinv = tail_head(po, on_act=True)
                    for f in tail_steps(sl, xe8, rinv, po=po):
                        f()

    nc.compile()
    _cache["nc"] = nc
    return nc


def _prep_maps(x, gn_w, gn_b, qkv_w, qkv_b, proj_w, proj_b):
    """Host-side sharding + layout prep. Returns list of 8 in_maps."""
    import ml_dtypes
    fp8 = ml_dtypes.float8_e4m3
    bf16 = ml_dtypes.bfloat16
    x = np.asarray(x, np.float32)
    qkv_w = np.asarray(qkv_w, np.float32)
    qkv_b = np.asarray(qkv_b, np.float32)
    proj_w = np.asarray(proj_w, np.float32)
    proj_b = np.asarray(proj_b, np.float32)
    gn_w = np.asarray(gn_w, np.float32)
    gn_b = np.asarray(gn_b, np.float32)

    def chunked(a):  # [256, ...] -> [128, 2, ...]
        return np.ascontiguousarray(a.reshape(2, 128, *a.shape[1:]).transpose(
            1, 0, *range(2, a.ndim + 1)))

    wq = chunked(qkv_w[0:C].T.copy()).astype(bf16)       # [c_in, 2, c_out]
    wvf = chunked(qkv_w[2 * C:3 * C].T.copy())           # [c_in, 2, c_out]
    wpf = chunked(proj_w.T.copy())                       # [c_in, 2, c_out]
    wvpf = np.concatenate([wvf, wpf], axis=2).astype(bf16)
    wkTb = chunked(qkv_w[C:2 * C].copy()).astype(bf16)   # [c_out, 2, c_in]
    wv88 = np.concatenate([wvf, wpf], axis=2).astype(fp8)
    rbias = proj_w @ qkv_b[2 * C:3 * C] + proj_b   # v-bias fold + proj bias
    kb_unused = np.zeros(C, np.float32)
    smalls = np.stack([qkv_b[0:C], kb_unused, gn_w, gn_b, rbias], axis=1)

    cidx = np.arange(C)
    ag_full = (cidx[:, None] // CPG == np.arange(G)[None, :]).astype(np.float32)
    ag = ag_full / (CPG * HW)                       # carries 1/(8*4096)
    hdr = chunked(np.concatenate([smalls, ag], axis=1))  # [128, 2, 37]
    bg_full = ag_full * gn_w[:, None]               # carries gn_w
    bg = np.ascontiguousarray(
        bg_full.reshape(2, 128, G).transpose(2, 0, 1))  # [G, 2, 128]
    eye = np.zeros((128, 132), np.float32)
    eye[np.arange(128), np.arange(128)] = 129.0
    eye = eye.astype(bf16)

    maps = []
    for core in range(8):
        b, half = core // 2, core % 2
        xf = x[b].reshape(C, HW)
        if half == 1:   # own query half first (key order is irrelevant
            xf = np.concatenate([xf[:, NH:], xf[:, :NH]], axis=1)
        xf = np.ascontiguousarray(xf)
        xhm = xf[:, 0:NH]                            # own half = residual
        x8c = chunked(xf).astype(fp8)
        # xT8e: [pix128, MB, XW]: [x_c0 | ones | pad | x_c1 | ones | pad]
        xT = np.ascontiguousarray(
            xf.T.reshape(MB, 128, C).transpose(1, 0, 2)).astype(fp8)
        xT8e = np.zeros((128, MB, XW), fp8)
        xT8e[:, :, 0:128] = xT[:, :, 0:128]
        xT8e[:, :, 128] = np.float32(1.0)
        xT8e[:, :, C1OFF:C1OFF + 128] = xT[:, :, 128:256]
        xT8e[:, :, C1OFF + 128] = np.float32(1.0)
        maps.append({
            "x8": x8c,
            "xT8": xT8e, "xh": chunked(xhm).astype(bf16),
            "wq": wq, "wvpf": wvpf,
            "wkTb": wkTb, "wv88": wv88,
            "hdr": hdr, "bg": bg, "eye": eye,
        })
    return maps


def kernel(x, gn_w, gn_b, qkv_w, qkv_b, proj_w, proj_b):
    import concourse.bass_utils as bu
    nc = build_nc()
    maps = _prep_maps(x, gn_w, gn_b, qkv_w, qkv_b, proj_w, proj_b)
    res = bu.run_bass_kernel_spmd(nc, maps, core_ids=list(range(8)))
    out = np.empty((B, C, HW), np.float32)
    for core in range(8):
        b, half = core // 2, core % 2
        o = res.results[core]["out"]                # [128, 2, NH]
        out[b, :, half * NH:(half + 1) * NH] = \
            o.transpose(1, 0, 2).reshape(C, NH)
    return out.reshape(B, C, 64, 64)
